# revision 1
# baseline (speedup 1.0000x reference)
"""Trainium2 Bass kernel for nn_F0Predictor (conv stack + LSTM decode), 8-core data-parallel.

Contract: kernel(**inputs) takes the FULL unsharded inputs (as produced by
setup_inputs()) and returns the full [128, num_steps, 2] float32 output.
Internally: batch is sharded 8 ways (16 per NeuronCore), weights replicated,
compute in bf16 with fp32 PSUM accumulation. No collectives.
"""
import numpy as np
import ml_dtypes

import concourse.bass as bass
import concourse.tile as tile
from concourse import bacc, mybir
from concourse.bass_utils import run_bass_kernel_spmd

BF = mybir.dt.bfloat16
F32 = mybir.dt.float32
BF_NP = ml_dtypes.bfloat16

NCORES = 8
BC = 16          # batch per core
GOFF = [0, 512, 1536, 1024]   # our gate order (i, f, o, g) -> torch row offset
Sigmoid = mybir.ActivationFunctionType.Sigmoid
Tanh = mybir.ActivationFunctionType.Tanh
Relu = mybir.ActivationFunctionType.Relu

_CACHE = {}
DEBUG = False


# --------------------------------------------------------------------------
# host-side prep (numpy): weight layout transforms, batch sharding
# --------------------------------------------------------------------------

def _prep(inp):
    f32 = np.float32
    P = {}
    x = np.asarray(inp["x"], f32).reshape(128, 8192)
    x_pad = np.zeros((128, 8224), f32)
    x_pad[:, 16:8208] = x
    T0 = np.stack([x_pad[:, k + 1: k + 1 + 8192: 4] for k in range(31)], 0)  # [31,128,2048]
    P["t0_full"] = T0.astype(BF_NP)

    w0 = np.asarray(inp["cw0"], f32)
    P["w0T"] = w0[:, 0, :].T.astype(BF_NP).copy()                 # [31, 64]
    P["cb0"] = np.asarray(inp["cb0"], f32).reshape(64, 1).copy()

    w1 = np.asarray(inp["cw1"], f32)
    w1p = np.zeros((128, 16, 128), f32)               # [r, kp, co]
    for k in range(16):
        w1p[0:64, k, :] = w1[:, :, 2 * k].T
        if 2 * k + 1 <= 30:
            w1p[64:128, k, :] = w1[:, :, 2 * k + 1].T
    P["w1p"] = w1p.astype(BF_NP)
    P["cb1"] = np.asarray(inp["cb1"], f32).reshape(128, 1).copy()

    w2 = np.asarray(inp["cw2"], f32)
    w2T = np.zeros((128, 31, 2, 128), f32)                        # [r, k, cc, co]
    for k in range(31):
        for cc in range(2):
            w2T[:, k, cc, :] = w2[128 * cc:128 * cc + 128, :, k].T
    P["w2T"] = w2T.astype(BF_NP)
    P["cb2"] = np.ascontiguousarray(np.asarray(inp["cb2"], f32).reshape(2, 128).T)

    w3 = np.asarray(inp["cw3"], f32)
    w3T = np.zeros((128, 31, 2, 4, 128), f32)                     # [r, k, ci, cc, co]
    for k in range(31):
        for ci in range(2):
            for cc in range(4):
                w3T[:, k, ci, cc, :] = w3[128 * cc:128 * cc + 128, 128 * ci:128 * ci + 128, k].T
    P["w3T"] = w3T.astype(BF_NP)
    P["cb3"] = np.ascontiguousarray(np.asarray(inp["cb3"], f32).reshape(4, 128).T)

    w4 = np.asarray(inp["cw4"], f32)
    w4R = np.zeros((31, 4, 128, 1024), f32)                       # [k, ci, r, co]
    for k in range(31):
        for ci in range(4):
            w4R[k, ci] = w4[:, 128 * ci:128 * ci + 128, k].T
    P["w4R"] = w4R.astype(BF_NP)
    P["cb4"] = np.asarray(inp["cb4"], f32).reshape(1, 1024).astype(BF_NP).copy()

    phw = np.asarray(inp["ph_w"], f32)
    pcw = np.asarray(inp["pc_w"], f32)
    pwT = np.zeros((64, 128, 2, 4, 128), f32)                     # [kk, r, s, hc, uu]
    for kk in range(64):
        for hc in range(4):
            pwT[kk, :, 0, hc, :] = phw[128 * hc:128 * hc + 128, 128 * kk:128 * kk + 128].T
            pwT[kk, :, 1, hc, :] = pcw[128 * hc:128 * hc + 128, 128 * kk:128 * kk + 128].T
    P["pwT"] = pwT.astype(BF_NP)
    pb = np.zeros((1, 2, 4, 128), f32)
    pb[0, 0] = np.asarray(inp["ph_b"], f32).reshape(4, 128)
    pb[0, 1] = np.asarray(inp["pc_b"], f32).reshape(4, 128)
    P["pb"] = pb.astype(BF_NP)

    whh = np.asarray(inp["w_hh"], f32)
    wR = np.zeros((128, 4, 4, 4, 128), f32)                       # [r, kk, g, hc, uu]
    for kk in range(4):
        for g in range(4):
            for hc in range(4):
                wR[:, kk, g, hc, :] = whh[GOFF[g] + 128 * hc: GOFF[g] + 128 * hc + 128,
                                          128 * kk:128 * kk + 128].T
    P["wR"] = wR.astype(BF_NP)

    wih = np.asarray(inp["w_ih"], f32)
    embw = np.asarray(inp["emb_w"], f32)
    M = wih @ embw
    const0 = np.asarray(inp["b_ih"], f32) + np.asarray(inp["b_hh"], f32)
    consts = const0 + wih @ np.asarray(inp["emb_b"], f32)
    mRA = np.zeros((2, 2, 4, 4, 128), f32)   # rows (lf0col, const); [row, s, g, hc, uu]
    mRB = np.zeros((1, 2, 4, 4, 128), f32)   # row (uvcol)
    for g in range(4):
        for hc in range(4):
            sl = slice(GOFF[g] + 128 * hc, GOFF[g] + 128 * hc + 128)
            mRA[0, 1, g, hc, :] = M[sl, 0]
            mRA[1, 0, g, hc, :] = const0[sl]
            mRA[1, 1, g, hc, :] = consts[sl]
            mRB[0, 1, g, hc, :] = M[sl, 1]
    P["mRA"] = mRA.astype(BF_NP)
    P["mRB"] = mRB.astype(BF_NP)

    hwT = np.zeros((128, 4, 2), f32)
    for kk in range(4):
        hwT[:, kk, 0] = np.asarray(inp["lf0_w"], f32)[0, 128 * kk:128 * kk + 128]
        hwT[:, kk, 1] = np.asarray(inp["uv_w"], f32)[0, 128 * kk:128 * kk + 128]
    P["hwT"] = hwT.astype(BF_NP)
    P["hb2"] = np.array([[np.asarray(inp["lf0_b"], f32).reshape(-1)[0],
                          np.asarray(inp["uv_b"], f32).reshape(-1)[0]]], f32)
    P["i128"] = np.eye(128, dtype=BF_NP)
    return P


# --------------------------------------------------------------------------
# device program
# --------------------------------------------------------------------------

def _build(T):
    nc = bacc.Bacc("TRN2", target_bir_lowering=False, debug=False, num_devices=NCORES)

    d_t0 = nc.dram_tensor("t0", [31, BC, 2048], BF, kind="ExternalInput")
    d_w0 = nc.dram_tensor("w0T", [31, 64], BF, kind="ExternalInput")
    d_cb0 = nc.dram_tensor("cb0", [64, 1], F32, kind="ExternalInput")
    d_w1 = nc.dram_tensor("w1p", [128, 16, 128], BF, kind="ExternalInput")
    d_cb1 = nc.dram_tensor("cb1", [128, 1], F32, kind="ExternalInput")
    d_w2 = nc.dram_tensor("w2T", [128, 31, 2, 128], BF, kind="ExternalInput")
    d_cb2 = nc.dram_tensor("cb2", [128, 2], F32, kind="ExternalInput")
    d_w3 = nc.dram_tensor("w3T", [128, 31, 2, 4, 128], BF, kind="ExternalInput")
    d_cb3 = nc.dram_tensor("cb3", [128, 4], F32, kind="ExternalInput")
    d_w4 = nc.dram_tensor("w4R", [31, 4, 128, 1024], BF, kind="ExternalInput")
    d_cb4 = nc.dram_tensor("cb4", [1, 1024], BF, kind="ExternalInput")
    d_pw = nc.dram_tensor("pwT", [64, 128, 2, 4, 128], BF, kind="ExternalInput")
    d_pb = nc.dram_tensor("pb", [1, 2, 4, 128], BF, kind="ExternalInput")
    d_wR = nc.dram_tensor("wR", [128, 4, 4, 4, 128], BF, kind="ExternalInput")
    d_mRA = nc.dram_tensor("mRA", [2, 2, 4, 4, 128], BF, kind="ExternalInput")
    d_mRB = nc.dram_tensor("mRB", [1, 2, 4, 4, 128], BF, kind="ExternalInput")
    d_oinitA = nc.dram_tensor("oinitA", [2, 16 * (T + 1)], BF, kind="ExternalInput")
    d_oinitB = nc.dram_tensor("oinitB", [1, 16 * (T + 1)], BF, kind="ExternalInput")
    d_hwT = nc.dram_tensor("hwT", [128, 4, 2], BF, kind="ExternalInput")
    d_hb2 = nc.dram_tensor("hb2", [1, 2], F32, kind="ExternalInput")
    d_i128 = nc.dram_tensor("i128", [128, 128], BF, kind="ExternalInput")
    d_out = nc.dram_tensor("out", [2, T, 16], F32, kind="ExternalOutput")
    dbg = {}
    if DEBUG:
        for nm, shp, dt in [("d_act1", [128, BC, 543], BF), ("d_act3", [128, BC, 63], BF),
                            ("d_out4T", [128, 1024], BF), ("d_hfT", [128, 1024], BF),
                            ("d_Hb0", [128, 128], BF), ("d_C0", [128, 128], F32),
                            ("d_hTT", [128, 128], BF), ("d_sifo", [128, 384], F32)]:
            dbg[nm] = nc.dram_tensor(nm, shp, dt, kind="ExternalOutput")

    from contextlib import ExitStack
    with tile.TileContext(nc) as tc, ExitStack() as top:
        const_pool = top.enter_context(tc.tile_pool(name="const", bufs=1))
        i128t = const_pool.tile([128, 128], BF)
        nc.sync.dma_start(i128t[:], d_i128.ap())
        hb2t = const_pool.tile([1, 2], F32)
        nc.sync.dma_start(hb2t[:], d_hb2.ap())

        # persistent activations for the conv chain
        act1_pool = top.enter_context(tc.tile_pool(name="act1", bufs=1))
        act2_pool = top.enter_context(tc.tile_pool(name="act2", bufs=1))
        act3_pool = top.enter_context(tc.tile_pool(name="act3", bufs=1))
        out4_pool = top.enter_context(tc.tile_pool(name="out4", bufs=1))

        act1 = act1_pool.tile([128, BC, 543], BF)
        nc.gpsimd.memset(act1[:], 0.0)
        act2 = [act2_pool.tile([128, BC, 159], BF, name=f"act2_{i}", tag=f"act2_{i}") for i in range(2)]
        for t_ in act2:
            nc.gpsimd.memset(t_[:], 0.0)
        act3 = [act3_pool.tile([128, BC, 63], BF, name=f"act3_{i}", tag=f"act3_{i}") for i in range(4)]
        for t_ in act3:
            nc.gpsimd.memset(t_[:], 0.0)
        out4T = out4_pool.tile([128, 1024], BF)

        # ---------------- L0 + L1 (own pools, freed after) ----------------
        with ExitStack() as es01:
            p01 = es01.enter_context(tc.tile_pool(name="p01", bufs=1))
            ps01 = es01.enter_context(tc.tile_pool(name="ps01", bufs=2, space="PSUM"))
            t0t = p01.tile([31, BC, 2048], BF)
            nc.sync.dma_start(t0t[:], d_t0.ap())
            w0t = p01.tile([31, 64], BF)
            nc.sync.dma_start(w0t[:], d_w0.ap())
            cb0t = p01.tile([64, 1], F32)
            nc.sync.dma_start(cb0t[:], d_cb0.ap())
            act0 = p01.tile([128, BC, 2079], BF)
            nc.gpsimd.memset(act0[:], 0.0)

            for bg in range(4):
                for lc in range(16):
                    p = ps01.tile([64, 4, 128], F32, name="l0ps", tag="l0ps")
                    nc.tensor.matmul(p[:], w0t[:],
                                     t0t[:, 4 * bg:4 * bg + 4, 128 * lc:128 * lc + 128],
                                     start=True, stop=True)
                    nc.scalar.activation(
                        act0[0:64, 4 * bg:4 * bg + 4, 15 + 128 * lc:15 + 128 * lc + 128],
                        p[:], Relu, bias=cb0t[:])
            # duplicate shifted by +1 element into partitions 64..127
            nc.sync.dma_start(act0[64:128, :, 0:2078], act0[0:64, :, 1:2079])

            w1t = p01.tile([128, 16, 128], BF)
            nc.sync.dma_start(w1t[:], d_w1.ap())
            cb1t = p01.tile([128, 1], F32)
            nc.sync.dma_start(cb1t[:], d_cb1.ap())

            for bg in range(4):
                for lc in range(4):
                    p1 = ps01.tile([128, 4, 128], F32, name="l1ps", tag="l1ps", bufs=4)
                    for kp in range(16):
                        j0 = 2 * kp + 512 * lc
                        rhs = act0[:, 4 * bg:4 * bg + 4, j0: j0 + 512: 4]
                        nc.tensor.matmul(p1[:], w1t[:, kp, :], rhs,
                                         start=(kp == 0), stop=(kp == 15))
                    nc.scalar.activation(
                        act1[:, 4 * bg:4 * bg + 4, 15 + 128 * lc:15 + 128 * lc + 128],
                        p1[:], Relu, bias=cb1t[:])

        if DEBUG:
            nc.sync.dma_start(dbg["d_act1"].ap(), act1[:])
        # ---------------- L2 ----------------
        with ExitStack() as es2:
            p2p = es2.enter_context(tc.tile_pool(name="p2", bufs=1))
            ps2 = es2.enter_context(tc.tile_pool(name="ps2", bufs=1, space="PSUM"))
            w2t = p2p.tile([128, 31, 2, 128], BF)
            nc.sync.dma_start(w2t[:], d_w2.ap())
            cb2t = p2p.tile([128, 2], F32)
            nc.sync.dma_start(cb2t[:], d_cb2.ap())
            for cc in range(2):
                p2 = [ps2.tile([128, 4, 128], F32, name=f"l2ps_{bg}", tag=f"l2ps_{bg}") for bg in range(4)]
                for k in range(31):
                    for bg in range(4):
                        rhs = act1[:, 4 * bg:4 * bg + 4, k: k + 512: 4]
                        nc.tensor.matmul(p2[bg][:], w2t[:, k, cc, :], rhs,
                                         start=(k == 0), stop=(k == 30))
                for bg in range(4):
                    nc.scalar.activation(act2[cc][:, 4 * bg:4 * bg + 4, 15:143],
                                         p2[bg][:], Relu, bias=cb2t[:, cc:cc+1])

        # ---------------- L3 ----------------
        with ExitStack() as es3:
            p3p = es3.enter_context(tc.tile_pool(name="p3", bufs=1))
            ps3 = es3.enter_context(tc.tile_pool(name="ps3", bufs=2, space="PSUM"))
            w3t = p3p.tile([128, 31, 2, 4, 128], BF)
            nc.sync.dma_start(w3t[:], d_w3.ap())
            cb3t = p3p.tile([128, 4], F32)
            nc.sync.dma_start(cb3t[:], d_cb3.ap())
            for cc in range(4):
                p3 = ps3.tile([128, BC, 32], F32, name="l3ps", tag="l3ps")
                n = 0
                for ci in range(2):
                    for k in range(31):
                        rhs = act2[ci][:, :, k:k + 128:4]
                        nc.tensor.matmul(p3[:], w3t[:, k, ci, cc, :], rhs,
                                         start=(n == 0), stop=(n == 61))
                        n += 1
                nc.scalar.activation(act3[cc][:, :, 15:47], p3[:], Relu, bias=cb3t[:, cc:cc+1])

        # ---------------- L4 (weights moving) ----------------
        with ExitStack() as es4:
            p4p = es4.enter_context(tc.tile_pool(name="p4", bufs=8))
            p4c = es4.enter_context(tc.tile_pool(name="p4c", bufs=1))
            ps4 = es4.enter_context(tc.tile_pool(name="ps4", bufs=1, space="PSUM"))
            ones1 = p4c.tile([1, 128], BF)
            nc.gpsimd.memset(ones1[:], 1.0)
            cb4t = p4c.tile([1, 1024], BF)
            nc.sync.dma_start(cb4t[:], d_cb4.ap())
            PT = [ps4.tile([128, 512], F32, name=f"l4ps_{j}", tag=f"l4ps_{j}") for j in range(2)]
            for j in range(2):
                nc.tensor.matmul(PT[j][:], ones1[:, 0:128], cb4t[:, 512 * j:512 * j + 512],
                                 start=True, stop=False)
            for k in range(31):
                for ci in range(4):
                    w4c = p4p.tile([128, 1024], BF, name="w4c", tag="w4c")
                    nc.sync.dma_start(w4c[:], d_w4.ap()[k, ci])
                    imt = p4p.tile([128, 8, 16], BF, name="imt", tag="imt", bufs=4)
                    nc.vector.tensor_copy(
                        imt[:], act3[ci][:, :, k:k + 32:4].rearrange("p b l -> p l b"))
                    last = (k == 30 and ci == 3)
                    for j in range(2):
                        nc.tensor.matmul(PT[j][:], imt[:], w4c[:, 512 * j:512 * j + 512],
                                         start=False, stop=last)
            for j in range(2):
                nc.scalar.activation(out4T[:, 512 * j:512 * j + 512], PT[j][:], Relu)

        if DEBUG:
            nc.sync.dma_start(dbg["d_act3"].ap(), act3[0][:])
            nc.sync.dma_start(dbg["d_out4T"].ap(), out4T[:])
        # ---------------- transposes + projections ----------------
        lstm_pool = top.enter_context(tc.tile_pool(name="lstm", bufs=1))
        C = lstm_pool.tile([128, 128], F32)
        outA = lstm_pool.tile([2, 16 * (T + 1)], BF)   # rows (lf0, ones)
        outB = lstm_pool.tile([1, 16 * (T + 1)], BF)   # row (sig(uv))
        nc.sync.dma_start(outA[:], d_oinitA.ap())
        nc.sync.dma_start(outB[:], d_oinitB.ap())

        state_pool = top.enter_context(tc.tile_pool(name="state", bufs=2))
        ps_tr = top.enter_context(tc.tile_pool(name="ps_tr", bufs=2, space="PSUM"))

        with ExitStack() as esp:
            ppw = esp.enter_context(tc.tile_pool(name="ppw", bufs=8))
            ppc = esp.enter_context(tc.tile_pool(name="ppc", bufs=1))
            psp = esp.enter_context(tc.tile_pool(name="psp", bufs=1, space="PSUM"))
            hfT = ppc.tile([128, 1024], BF)
            # transpose out4T[l*16+b, co] -> hfT[:, 16*kk+b] (kk = l*8 + c8),
            # two l-values per [32,128] transpose (base partitions 0/32/64/96)
            for q in range(4):
                ptile = ps_tr.tile([128, 8, 2, 16], BF, name="trp2", tag="trp")
                for c8 in range(8):
                    nc.tensor.transpose(
                        ptile[:, c8, :, :],
                        out4T[32 * q:32 * q + 32, 128 * c8:128 * c8 + 128],
                        i128t[32 * q:32 * q + 32, 32 * q:32 * q + 32],
                        tile_position=(32 * q, 0))
                dst = hfT[:, 256 * q:256 * q + 256].rearrange(
                    "p (l cc b) -> p cc l b", l=2, cc=8, b=16)
                nc.scalar.copy(dst, ptile[:])

            onesb = ppc.tile([1, 16], BF)
            nc.gpsimd.memset(onesb[:], 1.0)
            pbt = ppc.tile([1, 2, 4, 128], BF)
            nc.sync.dma_start(pbt[:], d_pb.ap())
            psh = [psp.tile([128, 128], F32, name=f"psh_{s}", tag=f"psh_{s}") for s in range(2)]
            for s in range(2):
                for hc in range(4):
                    nc.tensor.matmul(psh[s][32 * hc:32 * hc + BC, :], onesb[:],
                                     pbt[:, s, hc, :], start=True, stop=False,
                                     tile_position=(0, 32 * hc))
            for kk in range(64):
                pwc = ppw.tile([128, 2, 4, 128], BF, name="pwc", tag="pwc")
                nc.sync.dma_start(pwc[:], d_pw.ap()[kk])
                last = (kk == 63)
                for s in range(2):
                    for hc in range(4):
                        nc.tensor.matmul(psh[s][32 * hc:32 * hc + BC, :],
                                         hfT[:, 16 * kk:16 * kk + 16],
                                         pwc[:, s, hc, :], start=False, stop=last,
                                         tile_position=(0, 32 * hc))
            Hb0 = state_pool.tile([128, 128], BF, name="Hb", tag="Hb")
            nc.scalar.copy(Hb0[:], psh[0][:])
            nc.scalar.copy(C[:], psh[1][:])
            if DEBUG:
                nc.sync.dma_start(dbg["d_hfT"].ap(), hfT[:])
                nc.sync.dma_start(dbg["d_Hb0"].ap(), Hb0[:])
                nc.sync.dma_start(dbg["d_C0"].ap(), C[:])

        # ---------------- LSTM ----------------
        wRt = lstm_pool.tile([128, 4, 4, 4, 128], BF)
        nc.sync.dma_start(wRt[:], d_wR.ap())
        mRAt = lstm_pool.tile([2, 2, 4, 4, 128], BF)
        nc.sync.dma_start(mRAt[:], d_mRA.ap())
        mRBt = lstm_pool.tile([1, 2, 4, 4, 128], BF)
        nc.sync.dma_start(mRBt[:], d_mRB.ap())
        hwTt = lstm_pool.tile([128, 4, 2], BF)
        nc.sync.dma_start(hwTt[:], d_hwT.ap())

        ps_ifo = top.enter_context(tc.tile_pool(name="ps_ifo", bufs=2, space="PSUM"))
        ps_g = top.enter_context(tc.tile_pool(name="ps_g", bufs=2, space="PSUM"))
        ps_hd = top.enter_context(tc.tile_pool(name="ps_hd", bufs=1, space="PSUM"))
        work_pool = top.enter_context(tc.tile_pool(name="work", bufs=2))

        def trans_h(hb):
            pt = ps_tr.tile([128, 128], BF, name="trp", tag="trp")
            nc.tensor.transpose(pt[:], hb[:], i128t[:])
            hTT = state_pool.tile([128, 128], BF, name="hTT", tag="hTT")
            nc.scalar.copy(hTT[:], pt[:])
            return hTT

        hTT = trans_h(Hb0)
        if DEBUG:
            nc.sync.dma_start(dbg["d_hTT"].ap(), hTT[:])
        for t in range(T):
            pifo = ps_ifo.tile([128, 384], F32, name="pifo", tag="pifo")
            pg = ps_g.tile([128, 128], F32, name="pg", tag="pg")
            s_idx = 0 if t == 0 else 1
            SA = outA[:, 16 * t:16 * t + 16]
            SB = outB[:, 16 * t:16 * t + 16]
            for hc in range(4):
                tp = (0, 32 * hc)
                for kk in range(4):
                    lhs = hTT[:, 32 * kk:32 * kk + 16]
                    nc.tensor.matmul(pifo[32 * hc:32 * hc + BC, :], lhs,
                                     wRt[:, kk, 0:3, hc, :], start=(kk == 0),
                                     stop=False, tile_position=tp)
                    nc.tensor.matmul(pg[32 * hc:32 * hc + BC, :], lhs,
                                     wRt[:, kk, 3, hc, :], start=(kk == 0),
                                     stop=False, tile_position=tp)
                nc.tensor.matmul(pifo[32 * hc:32 * hc + BC, :], SA,
                                 mRAt[:, s_idx, 0:3, hc, :], start=False, stop=False,
                                 tile_position=tp)
                nc.tensor.matmul(pg[32 * hc:32 * hc + BC, :], SA,
                                 mRAt[:, s_idx, 3, hc, :], start=False, stop=False,
                                 tile_position=tp)
                nc.tensor.matmul(pifo[32 * hc:32 * hc + BC, :], SB,
                                 mRBt[:, s_idx, 0:3, hc, :], start=False, stop=True,
                                 tile_position=tp)
                nc.tensor.matmul(pg[32 * hc:32 * hc + BC, :], SB,
                                 mRBt[:, s_idx, 3, hc, :], start=False, stop=True,
                                 tile_position=tp)
            sifo = work_pool.tile([128, 384], F32, name="sifo", tag="sifo")
            nc.scalar.activation(sifo[:], pifo[:], Sigmoid)
            if DEBUG and t == 0:
                nc.sync.dma_start(dbg["d_sifo"].ap(), sifo[:])
            g = work_pool.tile([128, 128], F32, name="g", tag="g")
            nc.scalar.activation(g[:], pg[:], Tanh)
            t1 = work_pool.tile([128, 128], F32, name="t1", tag="t1")
            nc.vector.tensor_mul(t1[:], sifo[:, 128:256], C[:])
            t2 = work_pool.tile([128, 128], F32, name="t2", tag="t2")
            nc.vector.tensor_mul(t2[:], sifo[:, 0:128], g[:])
            nc.vector.tensor_add(C[:], t1[:], t2[:])
            tch = work_pool.tile([128, 128], F32, name="tch", tag="tch")
            nc.scalar.activation(tch[:], C[:], Tanh)
            hb = state_pool.tile([128, 128], BF, name="Hb", tag="Hb")
            nc.vector.tensor_mul(hb[:], sifo[:, 256:384], tch[:])
            hTT = trans_h(hb)
            phl = ps_hd.tile([1, 16], F32, name="phl", tag="phl")
            phu = ps_hd.tile([1, 16], F32, name="phu", tag="phu")
            for kk in range(4):
                nc.tensor.matmul(phl[:], hwTt[:, kk, 0:1], hTT[:, 32 * kk:32 * kk + 16],
                                 start=(kk == 0), stop=(kk == 3))
                nc.tensor.matmul(phu[:], hwTt[:, kk, 1:2], hTT[:, 32 * kk:32 * kk + 16],
                                 start=(kk == 0), stop=(kk == 3))
            o0 = 16 * (t + 1)
            nc.vector.tensor_scalar_add(outA[0:1, o0:o0 + 16], phl[:], hb2t[:, 0:1])
            nc.scalar.activation(outB[0:1, o0:o0 + 16], phu[:], Sigmoid,
                                 bias=hb2t[:, 1:2])

        OFl = lstm_pool.tile([1, T, 16], F32)
        nc.scalar.copy(OFl[:], outA[0:1, 16:16 * (T + 1)].rearrange("p (t b) -> p t b", t=T))
        OFu = lstm_pool.tile([1, T, 16], F32)
        nc.scalar.copy(OFu[:], outB[0:1, 16:16 * (T + 1)].rearrange("p (t b) -> p t b", t=T))
        nc.sync.dma_start(d_out.ap()[0:1], OFl[:])
        nc.sync.dma_start(d_out.ap()[1:2], OFu[:])

    nc.compile()
    return nc


# --------------------------------------------------------------------------
# entry point
# --------------------------------------------------------------------------

def _in_maps(P, T):
    shared = {k: P[k] for k in ["w0T", "cb0", "w1p", "cb1", "w2T", "cb2", "w3T", "cb3",
                                "w4R", "cb4", "pwT", "pb", "wR", "mRA", "mRB", "hwT",
                                "hb2", "i128"]}
    oinitA = np.zeros((2, 16 * (T + 1)), BF_NP)
    oinitA[1, :] = 1.0
    shared["oinitA"] = oinitA
    shared["oinitB"] = np.zeros((1, 16 * (T + 1)), BF_NP)
    in_maps = []
    for c in range(NCORES):
        m = dict(shared)
        m["t0"] = np.ascontiguousarray(P["t0_full"][:, BC * c:BC * c + BC, :])
        in_maps.append(m)
    return in_maps


def kernel(**inputs):
    T = int(np.asarray(inputs["num_steps"]))
    if T not in _CACHE:
        _CACHE[T] = _build(T)
    nc = _CACHE[T]
    P = _prep(inputs)
    in_maps = _in_maps(P, T)
    res = run_bass_kernel_spmd(nc, in_maps, list(range(NCORES)))
    out = np.empty((128, T, 2), np.float32)
    for c in range(NCORES):
        out[BC * c:BC * c + BC] = res.results[c]["out"].transpose(2, 1, 0)
    return out



# revision 17
# speedup vs baseline: 1.9856x; 1.9856x over previous
"""Trainium2 Bass kernel for nn_F0Predictor (conv stack + LSTM decode), 8-core data-parallel.

Contract: kernel(**inputs) takes the FULL unsharded inputs (as produced by
setup_inputs()) and returns the full [128, num_steps, 2] float32 output.
Internally: batch is sharded 8 ways (16 per NeuronCore), weights replicated,
compute in bf16 with fp32 PSUM accumulation. No collectives.

LSTM step design (per core, batch 16):
- gates PSUM [128, 512]: 4 col strips (hc) x [i,f,o,2*g] columns; g weights
  pre-scaled x2 so one sigmoid computes tanh via tanh(x)=2*sigmoid(2x)-1.
- gate matmuls issued kk-outer / hc-inner so the 4 col strips stream
  concurrently on disjoint PE column groups.
- rank-3 update (lf0, ones, sig(uv)) merged into ONE stationary [3,16] per
  strip (vs 2 separate in the old version): 512 moving rows per strip.
- cell update: u=C*s_f (DVE TT), v=(s_g-0.5)*s_i (DVE STT), C=2v+u (DVE STT),
  tanh (ACT), h=s_o*tch (DVE TT).
- dummy matmuls with data deps on elementwise intermediates keep the PE HAM
  clock warm (2.4 GHz) across the per-step elementwise gap.
"""
import numpy as np
import ml_dtypes

import concourse.bass as bass
import concourse.tile as tile
from concourse import bacc, mybir
from concourse.bass_utils import run_bass_kernel_spmd

BF = mybir.dt.bfloat16
F32 = mybir.dt.float32
BF_NP = ml_dtypes.bfloat16

NCORES = 8
BC = 16          # batch per core
GOFF3 = [0, 512, 1536, 1024]   # our gate col order (i, f, o, g) -> torch row offset
Sigmoid = mybir.ActivationFunctionType.Sigmoid
Tanh = mybir.ActivationFunctionType.Tanh
Relu = mybir.ActivationFunctionType.Relu
ALU = mybir.AluOpType

_CACHE = {}
DEBUG = False


# --------------------------------------------------------------------------
# host-side prep (numpy): weight layout transforms, batch sharding
# --------------------------------------------------------------------------

def _prep(inp):
    f32 = np.float32
    P = {}
    x = np.asarray(inp["x"], f32).reshape(128, 8192)
    x_pad = np.zeros((128, 8240), f32)
    x_pad[:, 16:8208] = x
    # t0[k, b, l] = x_pad[b, 4l + k + 1], l in [0, 2052)
    T0 = np.stack([x_pad[:, k + 1: k + 1 + 8208: 4] for k in range(31)], 0)  # [31,128,2052]
    P["t0_full"] = T0.astype(BF_NP)

    w0 = np.asarray(inp["cw0"], f32)
    P["w0T"] = w0[:, 0, :].T.astype(BF_NP).copy()                 # [31, 64]
    cb0 = np.asarray(inp["cb0"], f32).reshape(64, 1)
    P["cb0"] = np.concatenate([cb0, cb0], 0).copy()               # [128, 1] (dup for 2 halves)

    w1 = np.asarray(inp["cw1"], f32)
    w1p = np.zeros((128, 16, 128), f32)               # [r, kp, co]
    for k in range(16):
        w1p[0:64, k, :] = w1[:, :, 2 * k].T
        if 2 * k + 1 <= 30:
            w1p[64:128, k, :] = w1[:, :, 2 * k + 1].T
    P["w1p"] = w1p.astype(BF_NP)
    P["cb1"] = np.asarray(inp["cb1"], f32).reshape(128, 1).copy()

    w2 = np.asarray(inp["cw2"], f32)
    w2T = np.zeros((128, 31, 2, 128), f32)                        # [r, k, cc, co]
    for k in range(31):
        for cc in range(2):
            w2T[:, k, cc, :] = w2[128 * cc:128 * cc + 128, :, k].T
    P["w2T"] = w2T.astype(BF_NP)
    P["cb2"] = np.ascontiguousarray(np.asarray(inp["cb2"], f32).reshape(2, 128).T)

    w3 = np.asarray(inp["cw3"], f32)
    w3T = np.zeros((128, 31, 2, 4, 128), f32)                     # [r, k, ci, cc, co]
    for k in range(31):
        for ci in range(2):
            for cc in range(4):
                w3T[:, k, ci, cc, :] = w3[128 * cc:128 * cc + 128, 128 * ci:128 * ci + 128, k].T
    P["w3T"] = w3T.astype(BF_NP)
    P["cb3"] = np.ascontiguousarray(np.asarray(inp["cb3"], f32).reshape(4, 128).T)

    w4 = np.asarray(inp["cw4"], f32)
    w4R = np.zeros((31, 4, 128, 1024), f32)                       # [k, ci, r, co]
    for k in range(31):
        for ci in range(4):
            w4R[k, ci] = w4[:, 128 * ci:128 * ci + 128, k].T
    P["w4R"] = w4R.astype(BF_NP)
    P["cb4"] = np.asarray(inp["cb4"], f32).reshape(1, 1024).astype(BF_NP).copy()

    phw = np.asarray(inp["ph_w"], f32)
    pcw = np.asarray(inp["pc_w"], f32)
    pwT = np.zeros((64, 128, 2, 4, 128), f32)                     # [kk, r, s, hc, uu]
    for kk in range(64):
        for hc in range(4):
            pwT[kk, :, 0, hc, :] = phw[128 * hc:128 * hc + 128, 128 * kk:128 * kk + 128].T
            pwT[kk, :, 1, hc, :] = pcw[128 * hc:128 * hc + 128, 128 * kk:128 * kk + 128].T
    P["pwT"] = pwT.astype(BF_NP)
    pb = np.zeros((1, 2, 4, 128), f32)
    pb[0, 0] = np.asarray(inp["ph_b"], f32).reshape(4, 128)
    pb[0, 1] = np.asarray(inp["pc_b"], f32).reshape(4, 128)
    P["pb"] = pb.astype(BF_NP)

    # LSTM recurrent weights with the lf0 rank-1 term FOLDED IN:
    #   W' = w_hh + M0 (x) lf0_w   (M0 = w_ih @ emb_w[:,0])
    # wR4[r, kk, hc, 128*g+u] = W'[GOFF3[g]+128*hc+u, 128*kk+r], g-block x2.
    wih = np.asarray(inp["w_ih"], f32)
    embw = np.asarray(inp["emb_w"], f32)
    M = wih @ embw                                                # [2048, 2]
    lf0w = np.asarray(inp["lf0_w"], f32).reshape(-1)              # [512]
    whh = np.asarray(inp["w_hh"], f32) + np.outer(M[:, 0], lf0w)
    wR4 = np.zeros((128, 4, 4, 512), f32)
    for kk in range(4):
        for hc in range(4):
            for g in range(4):
                blk = whh[GOFF3[g] + 128 * hc: GOFF3[g] + 128 * hc + 128,
                          128 * kk:128 * kk + 128].T
                wR4[:, kk, hc, 128 * g:128 * g + 128] = blk * (2.0 if g == 3 else 1.0)
    P["wR4"] = wR4.astype(BF_NP)

    # rank-2 term: rows (uvcol=M[:,1], const_s); g-cols x2.  consts at s=1
    # include M0*lf0_b (the constant part of the folded lf0).
    const0 = np.asarray(inp["b_ih"], f32) + np.asarray(inp["b_hh"], f32)
    lf0b = np.asarray(inp["lf0_b"], f32).reshape(-1)[0]
    consts = const0 + wih @ np.asarray(inp["emb_b"], f32) + M[:, 0] * lf0b
    mr2 = np.zeros((2, 2, 4, 512), f32)                           # [row, s, hc, 512]
    m0neg = np.zeros((1, 4, 512), f32)                            # t=0 correction rhs
    for hc in range(4):
        for g in range(4):
            sl = slice(GOFF3[g] + 128 * hc, GOFF3[g] + 128 * hc + 128)
            sc = 2.0 if g == 3 else 1.0
            dst = slice(128 * g, 128 * g + 128)
            mr2[0, 1, hc, dst] = M[sl, 1] * sc
            mr2[1, 0, hc, dst] = const0[sl] * sc
            mr2[1, 1, hc, dst] = consts[sl] * sc
            m0neg[0, hc, dst] = -M[sl, 0] * sc
    P["mr2"] = mr2.astype(BF_NP)
    P["m0neg"] = m0neg.astype(BF_NP)

    # head: hwT[r, kk, (lf0, uv)] -> two separate [1,16] psums
    hwT = np.zeros((128, 4, 2), f32)
    for kk in range(4):
        hwT[:, kk, 0] = lf0w[128 * kk:128 * kk + 128]
        hwT[:, kk, 1] = np.asarray(inp["uv_w"], f32)[0, 128 * kk:128 * kk + 128]
    P["hwT"] = hwT.astype(BF_NP)
    P["hbl"] = np.array([[lf0b]], f32)
    P["hbu"] = np.array([[np.asarray(inp["uv_b"], f32).reshape(-1)[0]]], f32)
    P["i128"] = np.eye(128, dtype=BF_NP)
    return P


# --------------------------------------------------------------------------
# device program
# --------------------------------------------------------------------------

def _build(T):
    nc = bacc.Bacc("TRN2", target_bir_lowering=False, debug=False, num_devices=NCORES)

    d_t0 = nc.dram_tensor("t0", [31, BC, 2052], BF, kind="ExternalInput")
    d_w0 = nc.dram_tensor("w0T", [31, 64], BF, kind="ExternalInput")
    d_cb0 = nc.dram_tensor("cb0", [128, 1], F32, kind="ExternalInput")
    d_w1 = nc.dram_tensor("w1p", [128, 16, 128], BF, kind="ExternalInput")
    d_cb1 = nc.dram_tensor("cb1", [128, 1], F32, kind="ExternalInput")
    d_w2 = nc.dram_tensor("w2T", [128, 31, 2, 128], BF, kind="ExternalInput")
    d_cb2 = nc.dram_tensor("cb2", [128, 2], F32, kind="ExternalInput")
    d_w3 = nc.dram_tensor("w3T", [128, 31, 2, 4, 128], BF, kind="ExternalInput")
    d_cb3 = nc.dram_tensor("cb3", [128, 4], F32, kind="ExternalInput")
    d_w4 = nc.dram_tensor("w4R", [31, 4, 128, 1024], BF, kind="ExternalInput")
    d_cb4 = nc.dram_tensor("cb4", [1, 1024], BF, kind="ExternalInput")
    d_pw = nc.dram_tensor("pwT", [64, 128, 2, 4, 128], BF, kind="ExternalInput")
    d_pb = nc.dram_tensor("pb", [1, 2, 4, 128], BF, kind="ExternalInput")
    d_wR = nc.dram_tensor("wR4", [128, 4, 4, 512], BF, kind="ExternalInput")
    d_mr = nc.dram_tensor("mr2", [2, 2, 4, 512], BF, kind="ExternalInput")
    d_m0n = nc.dram_tensor("m0neg", [1, 4, 512], BF, kind="ExternalInput")
    d_oinit = nc.dram_tensor("oinit", [2, 16 * (T + 1)], BF, kind="ExternalInput")
    d_hwT = nc.dram_tensor("hwT", [128, 4, 2], BF, kind="ExternalInput")
    d_hbl = nc.dram_tensor("hbl", [1, 1], F32, kind="ExternalInput")
    d_hbu = nc.dram_tensor("hbu", [1, 1], F32, kind="ExternalInput")
    d_i128 = nc.dram_tensor("i128", [128, 128], BF, kind="ExternalInput")
    d_out = nc.dram_tensor("out", [2, T, 16], F32, kind="ExternalOutput")
    dbg = {}
    if DEBUG:
        for nm, shp, dt in [("d_act1", [128, BC, 543], BF), ("d_act3", [128, BC, 63], BF),
                            ("d_out4T", [128, 1024], BF), ("d_hfT", [128, 1024], BF),
                            ("d_Hb0", [128, 128], BF), ("d_C0", [128, 128], F32),
                            ("d_hTT", [128, 128], BF), ("d_sifo", [128, 512], BF),
                            ("d_C1", [128, 128], F32), ("d_act0", [128, BC, 2079], BF)]:
            dbg[nm] = nc.dram_tensor(nm, shp, dt, kind="ExternalOutput")

    from contextlib import ExitStack
    with tile.TileContext(nc) as tc, ExitStack() as top:
        const_pool = top.enter_context(tc.tile_pool(name="const", bufs=1))
        i128t = const_pool.tile([128, 128], BF)
        nc.sync.dma_start(i128t[:], d_i128.ap())
        hblt = const_pool.tile([1, 1], F32)
        nc.sync.dma_start(hblt[:], d_hbl.ap())
        hbut = const_pool.tile([1, 1], F32)
        nc.sync.dma_start(hbut[:], d_hbu.ap())

        # LSTM weights: prefetch at the very top (overlaps the conv stack)
        lstm_pool = top.enter_context(tc.tile_pool(name="lstm", bufs=1))
        wRt = lstm_pool.tile([128, 4, 4, 512], BF)
        nc.sync.dma_start(wRt[:], d_wR.ap())
        C = lstm_pool.tile([128, 128], F32)

        # persistent activations for the conv chain; act1/act2 freed after L3
        act3_pool = top.enter_context(tc.tile_pool(name="act3", bufs=1))
        out4_pool = top.enter_context(tc.tile_pool(name="out4", bufs=1))
        act12 = ExitStack()
        act1_pool = act12.enter_context(tc.tile_pool(name="act1", bufs=1))
        act2_pool = act12.enter_context(tc.tile_pool(name="act2", bufs=1))

        act1 = act1_pool.tile([128, BC, 543], BF)
        nc.gpsimd.memset(act1[:, :, 0:16], 0.0)
        nc.gpsimd.memset(act1[:, :, 527:543], 0.0)
        act2 = [act2_pool.tile([128, BC, 159], BF, name=f"act2_{i}", tag=f"act2_{i}") for i in range(2)]
        for t_ in act2:
            nc.gpsimd.memset(t_[:, :, 0:16], 0.0)
            nc.gpsimd.memset(t_[:, :, 143:159], 0.0)
        act3 = [act3_pool.tile([128, BC, 63], BF, name=f"act3_{i}", tag=f"act3_{i}") for i in range(4)]
        for t_ in act3:
            nc.gpsimd.memset(t_[:, :, 0:16], 0.0)
            nc.gpsimd.memset(t_[:, :, 47:63], 0.0)
        out4T = out4_pool.tile([128, 1024], BF)

        # ---------------- L0 + L1 (own pools, freed after) ----------------
        with ExitStack() as es01:
            p01 = es01.enter_context(tc.tile_pool(name="p01", bufs=1))
            ps01 = es01.enter_context(tc.tile_pool(name="ps01", bufs=2, space="PSUM"))
            w0t = p01.tile([31, 64], BF)
            nc.sync.dma_start(w0t[:], d_w0.ap())
            cb0t = p01.tile([128, 1], F32)
            nc.sync.dma_start(cb0t[:], d_cb0.ap())
            t0p = es01.enter_context(tc.tile_pool(name="t0p", bufs=2))
            act0 = p01.tile([128, BC, 2079], BF)
            nc.gpsimd.memset(act0[:, :, 0:15], 0.0)
            nc.gpsimd.memset(act0[:, :, 2063:2079], 0.0)
            zz = p01.tile([128, 4, 128], BF)
            nc.gpsimd.memset(zz[:], 0.0)

            # L0: two col-strips: partitions 0:64 = y[c, l], 64:128 = y[c, l+1]
            for bg in range(4):
                t0c = t0p.tile([31, 4, 2052], BF, name="t0c", tag="t0c")
                nc.sync.dma_start(t0c[:], d_t0.ap()[:, 4 * bg:4 * bg + 4, :])
                for lc in range(16):
                    p = ps01.tile([128, 4, 128], F32, name="l0ps", tag="l0ps")
                    nc.tensor.matmul(p[0:64], w0t[:],
                                     t0c[:, :, 128 * lc:128 * lc + 128],
                                     start=True, stop=True, tile_position=(0, 0))
                    nc.tensor.matmul(p[64:128], w0t[:],
                                     t0c[:, :, 128 * lc + 1:128 * lc + 129],
                                     start=True, stop=True, tile_position=(0, 64))
                    dst = act0[:, 4 * bg:4 * bg + 4, 15 + 128 * lc:15 + 128 * lc + 128]
                    if lc % 2 == 0:
                        nc.scalar.activation(dst, p[:], Relu, bias=cb0t[:])
                    else:
                        nc.vector.scalar_tensor_tensor(dst, p[:], cb0t[:], zz[:],
                                                       ALU.add, ALU.max)

            w1t = p01.tile([128, 16, 128], BF)
            nc.sync.dma_start(w1t[:], d_w1.ap())
            cb1t = p01.tile([128, 1], F32)
            nc.sync.dma_start(cb1t[:], d_cb1.ap())

            for bg in range(4):
                for lc in range(4):
                    p1 = ps01.tile([128, 4, 128], F32, name="l1ps", tag="l1ps", bufs=4)
                    for kp in range(16):
                        j0 = 2 * kp + 512 * lc
                        rhs = act0[:, 4 * bg:4 * bg + 4, j0: j0 + 512: 4]
                        nc.tensor.matmul(p1[:], w1t[:, kp, :], rhs,
                                         start=(kp == 0), stop=(kp == 15))
                    nc.scalar.activation(
                        act1[:, 4 * bg:4 * bg + 4, 15 + 128 * lc:15 + 128 * lc + 128],
                        p1[:], Relu, bias=cb1t[:])

        if DEBUG:
            nc.sync.dma_start(dbg["d_act0"].ap(), act0[:])
            nc.sync.dma_start(dbg["d_act1"].ap(), act1[:])
        # ---------------- L2 (w3 prefetched during L2) ----------------
        es23 = ExitStack()
        p3p = es23.enter_context(tc.tile_pool(name="p3", bufs=1))
        w3t = p3p.tile([128, 31, 2, 4, 128], BF)
        cb3t = p3p.tile([128, 4], F32)
        with ExitStack() as es2:
            p2p = es2.enter_context(tc.tile_pool(name="p2", bufs=1))
            ps2 = es2.enter_context(tc.tile_pool(name="ps2", bufs=1, space="PSUM"))
            w2t = p2p.tile([128, 31, 2, 128], BF)
            nc.sync.dma_start(w2t[:], d_w2.ap())
            cb2t = p2p.tile([128, 2], F32)
            nc.sync.dma_start(cb2t[:], d_cb2.ap())
            # prefetch L3 weights while L2 computes
            nc.sync.dma_start(w3t[:], d_w3.ap())
            nc.sync.dma_start(cb3t[:], d_cb3.ap())
            for cc in range(2):
                p2 = [ps2.tile([128, 4, 128], F32, name=f"l2ps_{bg}", tag=f"l2ps_{bg}") for bg in range(4)]
                for k in range(31):
                    for bg in range(4):
                        rhs = act1[:, 4 * bg:4 * bg + 4, k: k + 512: 4]
                        nc.tensor.matmul(p2[bg][:], w2t[:, k, cc, :], rhs,
                                         start=(k == 0), stop=(k == 30))
                for bg in range(4):
                    nc.scalar.activation(act2[cc][:, 4 * bg:4 * bg + 4, 15:143],
                                         p2[bg][:], Relu, bias=cb2t[:, cc:cc+1])

        # ---------------- L3 ----------------
        with ExitStack() as es3:
            ps3 = es3.enter_context(tc.tile_pool(name="ps3", bufs=2, space="PSUM"))
            for cc in range(4):
                p3 = ps3.tile([128, BC, 32], F32, name="l3ps", tag="l3ps")
                n = 0
                for ci in range(2):
                    for k in range(31):
                        rhs = act2[ci][:, :, k:k + 128:4]
                        nc.tensor.matmul(p3[:], w3t[:, k, ci, cc, :], rhs,
                                         start=(n == 0), stop=(n == 61))
                        n += 1
                nc.scalar.activation(act3[cc][:, :, 15:47], p3[:], Relu, bias=cb3t[:, cc:cc+1])
        es23.close()    # free w3t (62 KB/partition)

        # ---------------- L4 (weights streaming, deep prefetch) ----------------
        act12.close()   # free act1/act2 SBUF for the proj-weight prefetch
        mrt = lstm_pool.tile([2, 2, 4, 512], BF)
        nc.sync.dma_start(mrt[:], d_mr.ap())
        m0nt = lstm_pool.tile([1, 4, 512], BF)
        nc.sync.dma_start(m0nt[:], d_m0n.ap())
        hwTt = lstm_pool.tile([128, 4, 2], BF)
        nc.sync.dma_start(hwTt[:], d_hwT.ap())
        outUO = lstm_pool.tile([2, 16 * (T + 1)], BF)  # rows (sig_uv, ones)
        nc.sync.dma_start(outUO[:], d_oinit.ap())
        outL = lstm_pool.tile([1, 16 * (T + 1)], BF)   # lf0 outputs (extraction only)
        lf0s = lstm_pool.tile([1, 16], BF)             # t=0 correction stationary
        state_pool = top.enter_context(tc.tile_pool(name="state", bufs=2))
        ps_tr = top.enter_context(tc.tile_pool(name="ps_tr", bufs=2, space="PSUM"))
        with ExitStack() as es4:
            p4p = es4.enter_context(tc.tile_pool(name="p4", bufs=12))
            p4c = es4.enter_context(tc.tile_pool(name="p4c", bufs=1))
            ps4 = es4.enter_context(tc.tile_pool(name="ps4", bufs=1, space="PSUM"))
            # prefetch proj weights during L4 (~14 MB)
            ppw = es4.enter_context(tc.tile_pool(name="ppw", bufs=48))
            pwcs = []
            for kk in range(64):
                pwc = ppw.tile([128, 2, 4, 128], BF, name="pwc", tag="pwc")
                nc.sync.dma_start(pwc[:], d_pw.ap()[kk])
                pwcs.append(pwc)
            ones1 = p4c.tile([1, 128], BF)
            nc.gpsimd.memset(ones1[:], 1.0)
            cb4t = p4c.tile([1, 1024], BF)
            nc.sync.dma_start(cb4t[:], d_cb4.ap())
            PT = [ps4.tile([128, 512], F32, name=f"l4ps_{j}", tag=f"l4ps_{j}") for j in range(2)]
            for j in range(2):
                nc.tensor.matmul(PT[j][:], ones1[:, 0:128], cb4t[:, 512 * j:512 * j + 512],
                                 start=True, stop=False)
            for k in range(31):
                for ci in range(4):
                    w4c = p4p.tile([128, 1024], BF, name="w4c", tag="w4c")
                    nc.sync.dma_start(w4c[:], d_w4.ap()[k, ci])
                    imt = p4p.tile([128, 8, 16], BF, name="imt", tag="imt", bufs=4)
                    nc.vector.tensor_copy(
                        imt[:], act3[ci][:, :, k:k + 32:4].rearrange("p b l -> p l b"))
                    last = (k == 30 and ci == 3)
                    for j in range(2):
                        nc.tensor.matmul(PT[j][:], imt[:], w4c[:, 512 * j:512 * j + 512],
                                         start=False, stop=last)
            for j in range(2):
                nc.scalar.activation(out4T[:, 512 * j:512 * j + 512], PT[j][:], Relu)

            if DEBUG:
                nc.sync.dma_start(dbg["d_act3"].ap(), act3[0][:])
                nc.sync.dma_start(dbg["d_out4T"].ap(), out4T[:])

            # ---------------- transpose out4 + projections ----------------
            with ExitStack() as esp:
                ppc = esp.enter_context(tc.tile_pool(name="ppc", bufs=1))
                psp = esp.enter_context(tc.tile_pool(name="psp", bufs=1, space="PSUM"))
                hfT = ppc.tile([128, 1024], BF)
                # transpose out4T[l*16+b, co] -> hfT[:, 16*kk+b] (kk = l*8 + c8),
                # two l-values per [32,128] transpose (base partitions 0/32/64/96)
                for q in range(4):
                    ptile = ps_tr.tile([128, 8, 2, 16], BF, name="trp2", tag="trp")
                    for c8 in range(8):
                        nc.tensor.transpose(
                            ptile[:, c8, :, :],
                            out4T[32 * q:32 * q + 32, 128 * c8:128 * c8 + 128],
                            i128t[32 * q:32 * q + 32, 32 * q:32 * q + 32],
                            tile_position=(32 * q, 0))
                    dst = hfT[:, 256 * q:256 * q + 256].rearrange(
                        "p (l cc b) -> p cc l b", l=2, cc=8, b=16)
                    nc.scalar.copy(dst, ptile[:])

                onesb = ppc.tile([1, 16], BF)
                nc.gpsimd.memset(onesb[:], 1.0)
                pbt = ppc.tile([1, 2, 4, 128], BF)
                nc.sync.dma_start(pbt[:], d_pb.ap())
                psh = [psp.tile([128, 128], F32, name=f"psh_{s}", tag=f"psh_{s}") for s in range(2)]
                for s in range(2):
                    for hc in range(4):
                        nc.tensor.matmul(psh[s][32 * hc:32 * hc + BC, :], onesb[:],
                                         pbt[:, s, hc, :], start=True, stop=False,
                                         tile_position=(0, 32 * hc))
                for kk in range(64):
                    last = (kk == 63)
                    for s in range(2):
                        for hc in range(4):
                            nc.tensor.matmul(psh[s][32 * hc:32 * hc + BC, :],
                                             hfT[:, 16 * kk:16 * kk + 16],
                                             pwcs[kk][:, s, hc, :], start=False, stop=last,
                                             tile_position=(0, 32 * hc))
                Hb0 = state_pool.tile([128, 128], BF, name="Hb", tag="Hb")
                nc.scalar.copy(Hb0[:], psh[0][:])
                nc.scalar.copy(C[:], psh[1][:])
                if DEBUG:
                    nc.sync.dma_start(dbg["d_hfT"].ap(), hfT[:])
                    nc.sync.dma_start(dbg["d_Hb0"].ap(), Hb0[:])
                    nc.sync.dma_start(dbg["d_C0"].ap(), C[:])

        # ---------------- LSTM ----------------
        ps_g = top.enter_context(tc.tile_pool(name="ps_g", bufs=2, space="PSUM"))
        ps_hd = top.enter_context(tc.tile_pool(name="ps_hd", bufs=1, space="PSUM"))
        ps_dum = top.enter_context(tc.tile_pool(name="ps_dum", bufs=1, space="PSUM"))
        work_pool = top.enter_context(tc.tile_pool(name="work", bufs=2))

        scratch = ps_dum.tile([128, 384], F32)

        def trans_h(hb):
            pt = ps_tr.tile([128, 128], BF, name="trp", tag="trp")
            nc.tensor.transpose(pt[:], hb[:], i128t[:])
            hTT = state_pool.tile([128, 128], BF, name="hTT", tag="hTT")
            nc.scalar.copy(hTT[:], pt[:])
            return hTT

        hTT = trans_h(Hb0)
        if DEBUG:
            nc.sync.dma_start(dbg["d_hTT"].ap(), hTT[:])

        for t in range(T):
            s_idx = 0 if t == 0 else 1
            SR = outUO[:, 16 * t:16 * t + 16]
            # head: lf0 / uv for THIS step's h (two base-0 [1,16] psums)
            phl = ps_hd.tile([1, 16], F32, name="phl", tag="phl")
            phu = ps_hd.tile([1, 16], F32, name="phu", tag="phu")
            for kk in range(4):
                nc.tensor.matmul(phl[:], hwTt[:, kk, 0:1], hTT[:, 32 * kk:32 * kk + 16],
                                 start=(kk == 0), stop=(kk == 3))
                nc.tensor.matmul(phu[:], hwTt[:, kk, 1:2], hTT[:, 32 * kk:32 * kk + 16],
                                 start=(kk == 0), stop=(kk == 3))
            o0 = 16 * t
            nc.vector.tensor_scalar_add(outL[0:1, o0:o0 + 16], phl[:], hblt[0:1, 0:1])
            nc.scalar.activation(outUO[0:1, o0:o0 + 16], phu[:], Sigmoid,
                                 bias=hbut[0:1, 0:1])
            if t == 0:
                nc.vector.tensor_scalar_add(lf0s[:], phl[:], 0.0)

            # gates: kk-outer, hc-inner for col-strip concurrency
            pifo = ps_g.tile([128, 512], F32, name="pifo", tag="pifo")
            for kk in range(4):
                lhs = hTT[:, 32 * kk:32 * kk + 16]
                for hc in range(4):
                    nc.tensor.matmul(pifo[32 * hc:32 * hc + BC, :], lhs,
                                     wRt[:, kk, hc, :], start=(kk == 0), stop=False,
                                     tile_position=(0, 32 * hc))
            for hc in range(4):
                nc.tensor.matmul(pifo[32 * hc:32 * hc + BC, :], SR,
                                 mrt[:, s_idx, hc, :], start=False,
                                 stop=(t != 0), tile_position=(0, 32 * hc))
            if t == 0:
                # cancel the folded M0*(lf0_w . h_s) term (x_0 is zero)
                for hc in range(4):
                    nc.tensor.matmul(pifo[32 * hc:32 * hc + BC, :], lf0s[:],
                                     m0nt[:, hc, :], start=False, stop=True,
                                     tile_position=(0, 32 * hc))
            # dummy matmul #1: keeps HAM busy right after the gate stream
            nc.tensor.matmul(scratch[:], i128t[:], wRt[:, 0, 0, 0:384],
                             start=True, stop=True)

            sifo = work_pool.tile([128, 512], BF, name="sifo", tag="sifo")
            nc.scalar.activation(sifo[:], pifo[:], Sigmoid)
            if DEBUG and t == 0:
                nc.sync.dma_start(dbg["d_sifo"].ap(), sifo[:])
            # dummy #2: data-dep on sifo -> runs mid-gap
            nc.tensor.matmul(scratch[:], i128t[:], sifo[:, 0:384],
                             start=True, stop=True)
            # cell update: C = s_f*C + s_i*(2*s_g-1)
            u = work_pool.tile([128, 128], F32, name="u", tag="u")
            nc.vector.tensor_mul(u[:], C[:], sifo[:, 128:256])
            vp = work_pool.tile([128, 128], BF, name="vp", tag="vp")
            nc.vector.scalar_tensor_tensor(vp[:], sifo[:, 384:512], -0.5,
                                           sifo[:, 0:128], ALU.add, ALU.mult)
            nc.vector.scalar_tensor_tensor(C[:], vp[:], 2.0, u[:], ALU.mult, ALU.add)
            tch = work_pool.tile([128, 128], BF, name="tch", tag="tch")
            nc.scalar.activation(tch[:], C[:], Tanh)
            if DEBUG and t == 0:
                nc.sync.dma_start(dbg["d_C1"].ap(), C[:])
            # dummy #3: data-dep on tch -> runs late in the gap
            nc.tensor.matmul(scratch[:, 0:128], i128t[:], tch[:],
                             start=True, stop=True)
            hb = state_pool.tile([128, 128], BF, name="Hb", tag="Hb")
            nc.vector.tensor_mul(hb[:], sifo[:, 256:384], tch[:])
            hTT = trans_h(hb)

        # outputs for steps 1..T live in slots 1..T; the loop wrote slots
        # 0..T-1, so one more head on the final hTT fills slot T.
        phl = ps_hd.tile([1, 16], F32, name="phl", tag="phl")
        phu = ps_hd.tile([1, 16], F32, name="phu", tag="phu")
        for kk in range(4):
            nc.tensor.matmul(phl[:], hwTt[:, kk, 0:1], hTT[:, 32 * kk:32 * kk + 16],
                             start=(kk == 0), stop=(kk == 3))
            nc.tensor.matmul(phu[:], hwTt[:, kk, 1:2], hTT[:, 32 * kk:32 * kk + 16],
                             start=(kk == 0), stop=(kk == 3))
        o0 = 16 * T
        nc.vector.tensor_scalar_add(outL[0:1, o0:o0 + 16], phl[:], hblt[0:1, 0:1])
        nc.scalar.activation(outUO[0:1, o0:o0 + 16], phu[:], Sigmoid,
                             bias=hbut[0:1, 0:1])

        OFl = lstm_pool.tile([1, T, 16], F32)
        nc.scalar.copy(OFl[:], outL[0:1, 16:16 * (T + 1)].rearrange("p (t b) -> p t b", t=T))
        OFu = lstm_pool.tile([1, T, 16], F32)
        nc.scalar.copy(OFu[:], outUO[0:1, 16:16 * (T + 1)].rearrange("p (t b) -> p t b", t=T))
        nc.sync.dma_start(d_out.ap()[0:1], OFl[:])
        nc.sync.dma_start(d_out.ap()[1:2], OFu[:])

    nc.compile()
    return nc


# --------------------------------------------------------------------------
# entry point
# --------------------------------------------------------------------------

def _in_maps(P, T):
    shared = {k: P[k] for k in ["w0T", "cb0", "w1p", "cb1", "w2T", "cb2", "w3T", "cb3",
                                "w4R", "cb4", "pwT", "pb", "wR4", "mr2", "m0neg", "hwT",
                                "hbl", "hbu", "i128"]}
    oinit = np.zeros((2, 16 * (T + 1)), BF_NP)
    oinit[1, :] = 1.0
    shared["oinit"] = oinit
    in_maps = []
    for c in range(NCORES):
        m = dict(shared)
        m["t0"] = np.ascontiguousarray(P["t0_full"][:, BC * c:BC * c + BC, :])
        in_maps.append(m)
    return in_maps


def kernel(**inputs):
    T = int(np.asarray(inputs["num_steps"]))
    if T not in _CACHE:
        _CACHE[T] = _build(T)
    nc = _CACHE[T]
    P = _prep(inputs)
    in_maps = _in_maps(P, T)
    res = run_bass_kernel_spmd(nc, in_maps, list(range(NCORES)))
    out = np.empty((128, T, 2), np.float32)
    for c in range(NCORES):
        out[BC * c:BC * c + BC] = res.results[c]["out"].transpose(2, 1, 0)
    return out


# revision 19
# speedup vs baseline: 2.3256x; 1.1712x over previous
"""Trainium2 Bass kernel for nn_F0Predictor (conv stack + LSTM decode), 8-core data-parallel.

Contract: kernel(**inputs) takes the FULL unsharded inputs (as produced by
setup_inputs()) and returns the full [128, num_steps, 2] float32 output.
Internally: batch is sharded 8 ways (16 per NeuronCore), weights replicated,
compute in bf16 with fp32 PSUM accumulation. No collectives.

LSTM step design (per core, batch 16):
- gates PSUM [128, 512]: 4 col strips (hc) x [i,f,o,2*g] columns; g weights
  pre-scaled x2 so one sigmoid computes tanh via tanh(x)=2*sigmoid(2x)-1.
- gate matmuls issued kk-outer / hc-inner so the 4 col strips stream
  concurrently on disjoint PE column groups.
- rank-3 update (lf0, ones, sig(uv)) merged into ONE stationary [3,16] per
  strip (vs 2 separate in the old version): 512 moving rows per strip.
- cell update: u=C*s_f (DVE TT), v=(s_g-0.5)*s_i (DVE STT), C=2v+u (DVE STT),
  tanh (ACT), h=s_o*tch (DVE TT).
- dummy matmuls with data deps on elementwise intermediates keep the PE HAM
  clock warm (2.4 GHz) across the per-step elementwise gap.
"""
import numpy as np
import ml_dtypes

import concourse.bass as bass
import concourse.tile as tile
from concourse import bacc, mybir
from concourse.bass_utils import run_bass_kernel_spmd

BF = mybir.dt.bfloat16
F32 = mybir.dt.float32
BF_NP = ml_dtypes.bfloat16

NCORES = 8
BC = 16          # batch per core
GOFF3 = [0, 512, 1536, 1024]   # our gate col order (i, f, o, g) -> torch row offset
Sigmoid = mybir.ActivationFunctionType.Sigmoid
Tanh = mybir.ActivationFunctionType.Tanh
Relu = mybir.ActivationFunctionType.Relu
ALU = mybir.AluOpType

_CACHE = {}
DEBUG = False


# --------------------------------------------------------------------------
# host-side prep (numpy): weight layout transforms, batch sharding
# --------------------------------------------------------------------------

def _prep(inp):
    f32 = np.float32
    P = {}
    x = np.asarray(inp["x"], f32).reshape(128, 8192)
    x_pad = np.zeros((128, 8240), f32)
    x_pad[:, 16:8208] = x
    # t0[k, b, l] = x_pad[b, 4l + k + 1], l in [0, 2052)
    T0 = np.stack([x_pad[:, k + 1: k + 1 + 8208: 4] for k in range(31)], 0)  # [31,128,2052]
    P["t0_full"] = T0.astype(BF_NP)

    w0 = np.asarray(inp["cw0"], f32)
    P["w0T"] = w0[:, 0, :].T.astype(BF_NP).copy()                 # [31, 64]
    cb0 = np.asarray(inp["cb0"], f32).reshape(64, 1)
    P["cb0"] = np.concatenate([cb0, cb0], 0).copy()               # [128, 1] (dup for 2 halves)

    w1 = np.asarray(inp["cw1"], f32)
    w1p = np.zeros((128, 16, 128), f32)               # [r, kp, co]
    for k in range(16):
        w1p[0:64, k, :] = w1[:, :, 2 * k].T
        if 2 * k + 1 <= 30:
            w1p[64:128, k, :] = w1[:, :, 2 * k + 1].T
    P["w1p"] = w1p.astype(BF_NP)
    P["cb1"] = np.asarray(inp["cb1"], f32).reshape(128, 1).copy()

    w2 = np.asarray(inp["cw2"], f32)
    w2T = np.zeros((128, 31, 2, 128), f32)                        # [r, k, cc, co]
    for k in range(31):
        for cc in range(2):
            w2T[:, k, cc, :] = w2[128 * cc:128 * cc + 128, :, k].T
    P["w2T"] = w2T.astype(BF_NP)
    P["cb2"] = np.ascontiguousarray(np.asarray(inp["cb2"], f32).reshape(2, 128).T)

    w3 = np.asarray(inp["cw3"], f32)
    w3T = np.zeros((128, 31, 2, 4, 128), f32)                     # [r, k, ci, cc, co]
    for k in range(31):
        for ci in range(2):
            for cc in range(4):
                w3T[:, k, ci, cc, :] = w3[128 * cc:128 * cc + 128, 128 * ci:128 * ci + 128, k].T
    P["w3T"] = w3T.astype(BF_NP)
    P["cb3"] = np.ascontiguousarray(np.asarray(inp["cb3"], f32).reshape(4, 128).T)

    w4 = np.asarray(inp["cw4"], f32)
    w4R = np.zeros((31, 4, 128, 1024), f32)                       # [k, ci, r, co]
    for k in range(31):
        for ci in range(4):
            w4R[k, ci] = w4[:, 128 * ci:128 * ci + 128, k].T
    P["w4R"] = w4R.astype(BF_NP)
    P["cb4"] = np.asarray(inp["cb4"], f32).reshape(1, 1024).astype(BF_NP).copy()

    phw = np.asarray(inp["ph_w"], f32)
    pcw = np.asarray(inp["pc_w"], f32)
    pwT = np.zeros((64, 128, 2, 4, 128), f32)                     # [kk, r, s, hc, uu]
    for kk in range(64):
        for hc in range(4):
            pwT[kk, :, 0, hc, :] = phw[128 * hc:128 * hc + 128, 128 * kk:128 * kk + 128].T
            pwT[kk, :, 1, hc, :] = pcw[128 * hc:128 * hc + 128, 128 * kk:128 * kk + 128].T
    P["pwT"] = pwT.astype(BF_NP)
    pb = np.zeros((1, 2, 4, 128), f32)
    pb[0, 0] = np.asarray(inp["ph_b"], f32).reshape(4, 128)
    pb[0, 1] = np.asarray(inp["pc_b"], f32).reshape(4, 128)
    P["pb"] = pb.astype(BF_NP)

    # LSTM recurrent weights with the lf0 rank-1 term FOLDED IN:
    #   W' = w_hh + M0 (x) lf0_w   (M0 = w_ih @ emb_w[:,0])
    # wR4[r, kk, hc, 128*g+u] = W'[GOFF3[g]+128*hc+u, 128*kk+r], g-block x2.
    wih = np.asarray(inp["w_ih"], f32)
    embw = np.asarray(inp["emb_w"], f32)
    M = wih @ embw                                                # [2048, 2]
    lf0w = np.asarray(inp["lf0_w"], f32).reshape(-1)              # [512]
    whh = np.asarray(inp["w_hh"], f32) + np.outer(M[:, 0], lf0w)
    wR4 = np.zeros((128, 4, 4, 512), f32)
    for kk in range(4):
        for hc in range(4):
            for g in range(4):
                blk = whh[GOFF3[g] + 128 * hc: GOFF3[g] + 128 * hc + 128,
                          128 * kk:128 * kk + 128].T
                wR4[:, kk, hc, 128 * g:128 * g + 128] = blk * (2.0 if g == 3 else 1.0)
    P["wR4"] = wR4.astype(BF_NP)

    # rank-2 term: rows (uvcol=M[:,1], const_s); g-cols x2.  consts at s=1
    # include M0*lf0_b (the constant part of the folded lf0).
    const0 = np.asarray(inp["b_ih"], f32) + np.asarray(inp["b_hh"], f32)
    lf0b = np.asarray(inp["lf0_b"], f32).reshape(-1)[0]
    consts = const0 + wih @ np.asarray(inp["emb_b"], f32) + M[:, 0] * lf0b
    mr2 = np.zeros((2, 2, 4, 512), f32)                           # [row, s, hc, 512]
    m0neg = np.zeros((1, 4, 512), f32)                            # t=0 correction rhs
    for hc in range(4):
        for g in range(4):
            sl = slice(GOFF3[g] + 128 * hc, GOFF3[g] + 128 * hc + 128)
            sc = 2.0 if g == 3 else 1.0
            dst = slice(128 * g, 128 * g + 128)
            mr2[0, 1, hc, dst] = M[sl, 1] * sc
            mr2[1, 0, hc, dst] = const0[sl] * sc
            mr2[1, 1, hc, dst] = consts[sl] * sc
            m0neg[0, hc, dst] = -M[sl, 0] * sc
    P["mr2"] = mr2.astype(BF_NP)
    P["m0neg"] = m0neg.astype(BF_NP)

    # head: hwT[r, kk, (lf0, uv)] -> two separate [1,16] psums
    hwT = np.zeros((128, 4, 2), f32)
    for kk in range(4):
        hwT[:, kk, 0] = lf0w[128 * kk:128 * kk + 128]
        hwT[:, kk, 1] = np.asarray(inp["uv_w"], f32)[0, 128 * kk:128 * kk + 128]
    P["hwT"] = hwT.astype(BF_NP)
    P["hbl"] = np.array([[lf0b]], f32)
    P["hbu"] = np.array([[np.asarray(inp["uv_b"], f32).reshape(-1)[0]]], f32)
    P["i128"] = np.eye(128, dtype=BF_NP)
    return P


# --------------------------------------------------------------------------
# device program
# --------------------------------------------------------------------------

def _build(T):
    nc = bacc.Bacc("TRN2", target_bir_lowering=False, debug=False, num_devices=NCORES)

    d_t0 = nc.dram_tensor("t0", [31, BC, 2052], BF, kind="ExternalInput")
    d_w0 = nc.dram_tensor("w0T", [31, 64], BF, kind="ExternalInput")
    d_cb0 = nc.dram_tensor("cb0", [128, 1], F32, kind="ExternalInput")
    d_w1 = nc.dram_tensor("w1p", [128, 16, 128], BF, kind="ExternalInput")
    d_cb1 = nc.dram_tensor("cb1", [128, 1], F32, kind="ExternalInput")
    d_w2 = nc.dram_tensor("w2T", [128, 31, 2, 128], BF, kind="ExternalInput")
    d_cb2 = nc.dram_tensor("cb2", [128, 2], F32, kind="ExternalInput")
    d_w3 = nc.dram_tensor("w3T", [128, 31, 2, 4, 128], BF, kind="ExternalInput")
    d_cb3 = nc.dram_tensor("cb3", [128, 4], F32, kind="ExternalInput")
    d_w4 = nc.dram_tensor("w4R", [31, 4, 128, 1024], BF, kind="ExternalInput")
    d_cb4 = nc.dram_tensor("cb4", [1, 1024], BF, kind="ExternalInput")
    d_pw = nc.dram_tensor("pwT", [64, 128, 2, 4, 128], BF, kind="ExternalInput")
    d_pb = nc.dram_tensor("pb", [1, 2, 4, 128], BF, kind="ExternalInput")
    d_wR = nc.dram_tensor("wR4", [128, 4, 4, 512], BF, kind="ExternalInput")
    d_mr = nc.dram_tensor("mr2", [2, 2, 4, 512], BF, kind="ExternalInput")
    d_m0n = nc.dram_tensor("m0neg", [1, 4, 512], BF, kind="ExternalInput")
    d_oinit = nc.dram_tensor("oinit", [2, 16 * (T + 1)], BF, kind="ExternalInput")
    d_hwT = nc.dram_tensor("hwT", [128, 4, 2], BF, kind="ExternalInput")
    d_hbl = nc.dram_tensor("hbl", [1, 1], F32, kind="ExternalInput")
    d_hbu = nc.dram_tensor("hbu", [1, 1], F32, kind="ExternalInput")
    d_i128 = nc.dram_tensor("i128", [128, 128], BF, kind="ExternalInput")
    d_out = nc.dram_tensor("out", [2, T, 16], F32, kind="ExternalOutput")
    dbg = {}
    if DEBUG:
        for nm, shp, dt in [("d_act1", [128, BC, 543], BF), ("d_act3", [128, BC, 63], BF),
                            ("d_out4T", [128, 1024], BF), ("d_hfT", [128, 1024], BF),
                            ("d_Hb0", [128, 128], BF), ("d_C0", [128, 128], F32),
                            ("d_hTT", [128, 128], BF), ("d_sifo", [128, 512], BF),
                            ("d_C1", [128, 128], F32), ("d_act0", [128, BC, 2079], BF)]:
            dbg[nm] = nc.dram_tensor(nm, shp, dt, kind="ExternalOutput")

    from contextlib import ExitStack
    with tile.TileContext(nc) as tc, ExitStack() as top:
        const_pool = top.enter_context(tc.tile_pool(name="const", bufs=1))
        i128t = const_pool.tile([128, 128], BF)
        nc.sync.dma_start(i128t[:], d_i128.ap())
        hblt = const_pool.tile([1, 1], F32)
        nc.sync.dma_start(hblt[:], d_hbl.ap())
        hbut = const_pool.tile([1, 1], F32)
        nc.sync.dma_start(hbut[:], d_hbu.ap())

        # LSTM weights: prefetch at the very top (overlaps the conv stack)
        lstm_pool = top.enter_context(tc.tile_pool(name="lstm", bufs=1))
        wRt = lstm_pool.tile([128, 4, 4, 512], BF)
        nc.sync.dma_start(wRt[:], d_wR.ap())
        C = lstm_pool.tile([128, 128], F32)

        # persistent activations for the conv chain; act1/act2 freed after L3
        act3_pool = top.enter_context(tc.tile_pool(name="act3", bufs=1))
        out4_pool = top.enter_context(tc.tile_pool(name="out4", bufs=1))
        act12 = ExitStack()
        act1_pool = act12.enter_context(tc.tile_pool(name="act1", bufs=1))
        act2_pool = act12.enter_context(tc.tile_pool(name="act2", bufs=1))

        act1 = act1_pool.tile([128, BC, 543], BF)
        nc.gpsimd.memset(act1[:, :, 0:16], 0.0)
        nc.gpsimd.memset(act1[:, :, 527:543], 0.0)
        act2 = [act2_pool.tile([128, BC, 159], BF, name=f"act2_{i}", tag=f"act2_{i}") for i in range(2)]
        for t_ in act2:
            nc.gpsimd.memset(t_[:, :, 0:16], 0.0)
            nc.gpsimd.memset(t_[:, :, 143:159], 0.0)
        act3 = [act3_pool.tile([128, BC, 63], BF, name=f"act3_{i}", tag=f"act3_{i}") for i in range(4)]
        for t_ in act3:
            nc.gpsimd.memset(t_[:, :, 0:16], 0.0)
            nc.gpsimd.memset(t_[:, :, 47:63], 0.0)
        out4T = out4_pool.tile([128, 1024], BF)

        # ---------------- L0 + L1 (own pools, freed after) ----------------
        with ExitStack() as es01:
            p01 = es01.enter_context(tc.tile_pool(name="p01", bufs=1))
            ps01 = es01.enter_context(tc.tile_pool(name="ps01", bufs=2, space="PSUM"))
            w0t = p01.tile([31, 64], BF)
            nc.sync.dma_start(w0t[:], d_w0.ap())
            cb0t = p01.tile([128, 1], F32)
            nc.sync.dma_start(cb0t[:], d_cb0.ap())
            t0p = es01.enter_context(tc.tile_pool(name="t0p", bufs=2))
            act0 = p01.tile([128, BC, 2079], BF)
            nc.gpsimd.memset(act0[:, :, 0:15], 0.0)
            nc.gpsimd.memset(act0[:, :, 2063:2079], 0.0)
            zz = p01.tile([128, 4, 128], BF)
            nc.gpsimd.memset(zz[:], 0.0)

            # L0: two col-strips: partitions 0:64 = y[c, l], 64:128 = y[c, l+1]
            for bg in range(4):
                t0c = t0p.tile([31, 4, 2052], BF, name="t0c", tag="t0c")
                nc.sync.dma_start(t0c[:], d_t0.ap()[:, 4 * bg:4 * bg + 4, :])
                for lc in range(16):
                    p = ps01.tile([128, 4, 128], F32, name="l0ps", tag="l0ps")
                    nc.tensor.matmul(p[0:64], w0t[:],
                                     t0c[:, :, 128 * lc:128 * lc + 128],
                                     start=True, stop=True, tile_position=(0, 0))
                    nc.tensor.matmul(p[64:128], w0t[:],
                                     t0c[:, :, 128 * lc + 1:128 * lc + 129],
                                     start=True, stop=True, tile_position=(0, 64))
                    dst = act0[:, 4 * bg:4 * bg + 4, 15 + 128 * lc:15 + 128 * lc + 128]
                    if lc % 2 == 0:
                        nc.scalar.activation(dst, p[:], Relu, bias=cb0t[:])
                    else:
                        nc.vector.scalar_tensor_tensor(dst, p[:], cb0t[:], zz[:],
                                                       ALU.add, ALU.max)

            w1t = p01.tile([128, 16, 128], BF)
            nc.sync.dma_start(w1t[:], d_w1.ap())
            cb1t = p01.tile([128, 1], F32)
            nc.sync.dma_start(cb1t[:], d_cb1.ap())

            for bg in range(4):
                for lc in range(4):
                    p1 = ps01.tile([128, 4, 128], F32, name="l1ps", tag="l1ps", bufs=4)
                    for kp in range(16):
                        j0 = 2 * kp + 512 * lc
                        rhs = act0[:, 4 * bg:4 * bg + 4, j0: j0 + 512: 4]
                        nc.tensor.matmul(p1[:], w1t[:, kp, :], rhs,
                                         start=(kp == 0), stop=(kp == 15))
                    nc.scalar.activation(
                        act1[:, 4 * bg:4 * bg + 4, 15 + 128 * lc:15 + 128 * lc + 128],
                        p1[:], Relu, bias=cb1t[:])

        if DEBUG:
            nc.sync.dma_start(dbg["d_act0"].ap(), act0[:])
            nc.sync.dma_start(dbg["d_act1"].ap(), act1[:])
        # ---------------- L2 (w3 prefetched during L2) ----------------
        es23 = ExitStack()
        p3p = es23.enter_context(tc.tile_pool(name="p3", bufs=1))
        w3t = p3p.tile([128, 31, 2, 4, 128], BF)
        cb3t = p3p.tile([128, 4], F32)
        with ExitStack() as es2:
            p2p = es2.enter_context(tc.tile_pool(name="p2", bufs=1))
            ps2 = es2.enter_context(tc.tile_pool(name="ps2", bufs=1, space="PSUM"))
            w2t = p2p.tile([128, 31, 2, 128], BF)
            nc.sync.dma_start(w2t[:], d_w2.ap())
            cb2t = p2p.tile([128, 2], F32)
            nc.sync.dma_start(cb2t[:], d_cb2.ap())
            # prefetch L3 weights while L2 computes
            nc.sync.dma_start(w3t[:], d_w3.ap())
            nc.sync.dma_start(cb3t[:], d_cb3.ap())
            for cc in range(2):
                p2 = [ps2.tile([128, 4, 128], F32, name=f"l2ps_{bg}", tag=f"l2ps_{bg}") for bg in range(4)]
                for k in range(31):
                    for bg in range(4):
                        rhs = act1[:, 4 * bg:4 * bg + 4, k: k + 512: 4]
                        nc.tensor.matmul(p2[bg][:], w2t[:, k, cc, :], rhs,
                                         start=(k == 0), stop=(k == 30))
                for bg in range(4):
                    nc.scalar.activation(act2[cc][:, 4 * bg:4 * bg + 4, 15:143],
                                         p2[bg][:], Relu, bias=cb2t[:, cc:cc+1])

        # ---------------- L3 ----------------
        with ExitStack() as es3:
            ps3 = es3.enter_context(tc.tile_pool(name="ps3", bufs=2, space="PSUM"))
            for cc in range(4):
                p3 = ps3.tile([128, BC, 32], F32, name="l3ps", tag="l3ps")
                n = 0
                for ci in range(2):
                    for k in range(31):
                        rhs = act2[ci][:, :, k:k + 128:4]
                        nc.tensor.matmul(p3[:], w3t[:, k, ci, cc, :], rhs,
                                         start=(n == 0), stop=(n == 61))
                        n += 1
                nc.scalar.activation(act3[cc][:, :, 15:47], p3[:], Relu, bias=cb3t[:, cc:cc+1])
        es23.close()    # free w3t (62 KB/partition)

        # ---------------- L4 (weights streaming, deep prefetch) ----------------
        act12.close()   # free act1/act2 SBUF for the proj-weight prefetch
        mrt = lstm_pool.tile([2, 2, 4, 512], BF)
        nc.sync.dma_start(mrt[:], d_mr.ap())
        m0nt = lstm_pool.tile([1, 4, 512], BF)
        nc.sync.dma_start(m0nt[:], d_m0n.ap())
        hwTt = lstm_pool.tile([128, 4, 2], BF)
        nc.sync.dma_start(hwTt[:], d_hwT.ap())
        outUO = lstm_pool.tile([2, 16 * (T + 1)], BF)  # rows (sig_uv, ones)
        nc.sync.dma_start(outUO[:], d_oinit.ap())
        outL = lstm_pool.tile([1, 16 * (T + 1)], BF)   # lf0 outputs (extraction only)
        lf0s = lstm_pool.tile([1, 16], BF)             # t=0 correction stationary
        state_pool = top.enter_context(tc.tile_pool(name="state", bufs=2))
        ps_tr = top.enter_context(tc.tile_pool(name="ps_tr", bufs=2, space="PSUM"))
        with ExitStack() as es4:
            p4p = es4.enter_context(tc.tile_pool(name="p4", bufs=12))
            p4c = es4.enter_context(tc.tile_pool(name="p4c", bufs=1))
            ps4 = es4.enter_context(tc.tile_pool(name="ps4", bufs=1, space="PSUM"))
            # prefetch proj weights during L4 (~14 MB)
            ppw = es4.enter_context(tc.tile_pool(name="ppw", bufs=48))
            pwcs = []
            for kk in range(64):
                pwc = ppw.tile([128, 2, 4, 128], BF, name="pwc", tag="pwc")
                nc.sync.dma_start(pwc[:], d_pw.ap()[kk])
                pwcs.append(pwc)
            ones1 = p4c.tile([1, 128], BF)
            nc.gpsimd.memset(ones1[:], 1.0)
            cb4t = p4c.tile([1, 1024], BF)
            nc.sync.dma_start(cb4t[:], d_cb4.ap())
            PT = [ps4.tile([128, 512], F32, name=f"l4ps_{j}", tag=f"l4ps_{j}") for j in range(2)]
            for j in range(2):
                nc.tensor.matmul(PT[j][:], ones1[:, 0:128], cb4t[:, 512 * j:512 * j + 512],
                                 start=True, stop=False)
            for k in range(31):
                for ci in range(4):
                    w4c = p4p.tile([128, 1024], BF, name="w4c", tag="w4c")
                    nc.sync.dma_start(w4c[:], d_w4.ap()[k, ci])
                    imt = p4p.tile([128, 8, 16], BF, name="imt", tag="imt", bufs=4)
                    nc.vector.tensor_copy(
                        imt[:], act3[ci][:, :, k:k + 32:4].rearrange("p b l -> p l b"))
                    last = (k == 30 and ci == 3)
                    for j in range(2):
                        nc.tensor.matmul(PT[j][:], imt[:], w4c[:, 512 * j:512 * j + 512],
                                         start=False, stop=last)
            for j in range(2):
                nc.scalar.activation(out4T[:, 512 * j:512 * j + 512], PT[j][:], Relu)

            if DEBUG:
                nc.sync.dma_start(dbg["d_act3"].ap(), act3[0][:])
                nc.sync.dma_start(dbg["d_out4T"].ap(), out4T[:])

            # ---------------- transpose out4 + projections ----------------
            with ExitStack() as esp:
                ppc = esp.enter_context(tc.tile_pool(name="ppc", bufs=1))
                psp = esp.enter_context(tc.tile_pool(name="psp", bufs=1, space="PSUM"))
                hfT = ppc.tile([128, 1024], BF)
                # transpose out4T[l*16+b, co] -> hfT[:, 16*kk+b] (kk = l*8 + c8),
                # two l-values per [32,128] transpose (base partitions 0/32/64/96)
                for q in range(4):
                    ptile = ps_tr.tile([128, 8, 2, 16], BF, name="trp2", tag="trp")
                    for c8 in range(8):
                        nc.tensor.transpose(
                            ptile[:, c8, :, :],
                            out4T[32 * q:32 * q + 32, 128 * c8:128 * c8 + 128],
                            i128t[32 * q:32 * q + 32, 32 * q:32 * q + 32],
                            tile_position=(32 * q, 0))
                    dst = hfT[:, 256 * q:256 * q + 256].rearrange(
                        "p (l cc b) -> p cc l b", l=2, cc=8, b=16)
                    nc.scalar.copy(dst, ptile[:])

                onesb = ppc.tile([1, 16], BF)
                nc.gpsimd.memset(onesb[:], 1.0)
                pbt = ppc.tile([1, 2, 4, 128], BF)
                nc.sync.dma_start(pbt[:], d_pb.ap())
                psh = [psp.tile([128, 128], F32, name=f"psh_{s}", tag=f"psh_{s}") for s in range(2)]
                for s in range(2):
                    for hc in range(4):
                        nc.tensor.matmul(psh[s][32 * hc:32 * hc + BC, :], onesb[:],
                                         pbt[:, s, hc, :], start=True, stop=False,
                                         tile_position=(0, 32 * hc))
                for kk in range(64):
                    last = (kk == 63)
                    for s in range(2):
                        for hc in range(4):
                            nc.tensor.matmul(psh[s][32 * hc:32 * hc + BC, :],
                                             hfT[:, 16 * kk:16 * kk + 16],
                                             pwcs[kk][:, s, hc, :], start=False, stop=last,
                                             tile_position=(0, 32 * hc))
                Hb0 = state_pool.tile([128, 128], BF, name="Hb", tag="Hb")
                nc.scalar.copy(Hb0[:], psh[0][:])
                nc.scalar.copy(C[:], psh[1][:])
                if DEBUG:
                    nc.sync.dma_start(dbg["d_hfT"].ap(), hfT[:])
                    nc.sync.dma_start(dbg["d_Hb0"].ap(), Hb0[:])
                    nc.sync.dma_start(dbg["d_C0"].ap(), C[:])

        # ---------------- LSTM ----------------
        ps_g = top.enter_context(tc.tile_pool(name="ps_g", bufs=2, space="PSUM"))
        ps_hd = top.enter_context(tc.tile_pool(name="ps_hd", bufs=1, space="PSUM"))
        ps_dum = top.enter_context(tc.tile_pool(name="ps_dum", bufs=1, space="PSUM"))
        work_pool = top.enter_context(tc.tile_pool(name="work", bufs=2))

        scratch = ps_dum.tile([128, 512], F32)

        def trans_h(hb):
            pt = ps_tr.tile([128, 128], BF, name="trp", tag="trp")
            nc.tensor.transpose(pt[:], hb[:], i128t[:])
            hTT = state_pool.tile([128, 128], BF, name="hTT", tag="hTT")
            nc.vector.tensor_copy(hTT[:], pt[:])
            return hTT

        hTT = trans_h(Hb0)
        if DEBUG:
            nc.sync.dma_start(dbg["d_hTT"].ap(), hTT[:])

        for t in range(T):
            s_idx = 0 if t == 0 else 1
            SR = outUO[:, 16 * t:16 * t + 16]
            # head: lf0 / uv for THIS step's h (two base-0 [1,16] psums)
            phl = ps_hd.tile([1, 16], F32, name="phl", tag="phl")
            phu = ps_hd.tile([1, 16], F32, name="phu", tag="phu")
            for kk in range(4):
                nc.tensor.matmul(phl[:], hwTt[:, kk, 0:1], hTT[:, 32 * kk:32 * kk + 16],
                                 start=(kk == 0), stop=(kk == 3))
                nc.tensor.matmul(phu[:], hwTt[:, kk, 1:2], hTT[:, 32 * kk:32 * kk + 16],
                                 start=(kk == 0), stop=(kk == 3))
            o0 = 16 * t
            nc.vector.tensor_scalar_add(outL[0:1, o0:o0 + 16], phl[:], hblt[0:1, 0:1])
            nc.scalar.activation(outUO[0:1, o0:o0 + 16], phu[:], Sigmoid,
                                 bias=hbut[0:1, 0:1])
            if t == 0:
                nc.vector.tensor_scalar_add(lf0s[:], phl[:], 0.0)

            # gates: kk-outer, hc-inner for col-strip concurrency
            pifo = ps_g.tile([128, 512], F32, name="pifo", tag="pifo")
            for kk in range(4):
                lhs = hTT[:, 32 * kk:32 * kk + 16]
                for hc in range(4):
                    nc.tensor.matmul(pifo[32 * hc:32 * hc + BC, :], lhs,
                                     wRt[:, kk, hc, :], start=(kk == 0), stop=False,
                                     tile_position=(0, 32 * hc))
            for hc in range(4):
                nc.tensor.matmul(pifo[32 * hc:32 * hc + BC, :], SR,
                                 mrt[:, s_idx, hc, :], start=False,
                                 stop=(t != 0), tile_position=(0, 32 * hc))
            if t == 0:
                # cancel the folded M0*(lf0_w . h_s) term (x_0 is zero)
                for hc in range(4):
                    nc.tensor.matmul(pifo[32 * hc:32 * hc + BC, :], lf0s[:],
                                     m0nt[:, hc, :], start=False, stop=True,
                                     tile_position=(0, 32 * hc))
            # HAM gap-fill: back-to-back dummy matmuls keep the PE busy
            # (and thus at 2.4 GHz) across the elementwise gap.
            for dk in range(9):
                nc.tensor.matmul(scratch[:], i128t[:], wRt[:, dk % 4, 0, :],
                                 start=True, stop=True)

            sifo = work_pool.tile([128, 512], BF, name="sifo", tag="sifo")
            nc.scalar.activation(sifo[:], pifo[:], Sigmoid)
            if DEBUG and t == 0:
                nc.sync.dma_start(dbg["d_sifo"].ap(), sifo[:])
            nc.tensor.matmul(scratch[:], i128t[:], sifo[:],
                             start=True, stop=True)
            # cell update: C = s_f*C + s_i*(2*s_g-1)
            u = work_pool.tile([128, 128], F32, name="u", tag="u")
            nc.vector.tensor_mul(u[:], C[:], sifo[:, 128:256])
            vp = work_pool.tile([128, 128], BF, name="vp", tag="vp")
            nc.vector.scalar_tensor_tensor(vp[:], sifo[:, 384:512], -0.5,
                                           sifo[:, 0:128], ALU.add, ALU.mult)
            nc.vector.scalar_tensor_tensor(C[:], vp[:], 2.0, u[:], ALU.mult, ALU.add)
            nc.tensor.matmul(scratch[:, 0:128], i128t[:], vp[:],
                             start=True, stop=True)
            tch = work_pool.tile([128, 128], BF, name="tch", tag="tch")
            nc.scalar.activation(tch[:], C[:], Tanh)
            if DEBUG and t == 0:
                nc.sync.dma_start(dbg["d_C1"].ap(), C[:])
            nc.tensor.matmul(scratch[:, 0:128], i128t[:], tch[:],
                             start=True, stop=True)
            hb = state_pool.tile([128, 128], BF, name="Hb", tag="Hb")
            nc.vector.tensor_mul(hb[:], sifo[:, 256:384], tch[:])
            hTT = trans_h(hb)

        # outputs for steps 1..T live in slots 1..T; the loop wrote slots
        # 0..T-1, so one more head on the final hTT fills slot T.
        phl = ps_hd.tile([1, 16], F32, name="phl", tag="phl")
        phu = ps_hd.tile([1, 16], F32, name="phu", tag="phu")
        for kk in range(4):
            nc.tensor.matmul(phl[:], hwTt[:, kk, 0:1], hTT[:, 32 * kk:32 * kk + 16],
                             start=(kk == 0), stop=(kk == 3))
            nc.tensor.matmul(phu[:], hwTt[:, kk, 1:2], hTT[:, 32 * kk:32 * kk + 16],
                             start=(kk == 0), stop=(kk == 3))
        o0 = 16 * T
        nc.vector.tensor_scalar_add(outL[0:1, o0:o0 + 16], phl[:], hblt[0:1, 0:1])
        nc.scalar.activation(outUO[0:1, o0:o0 + 16], phu[:], Sigmoid,
                             bias=hbut[0:1, 0:1])

        OFl = lstm_pool.tile([1, T, 16], F32)
        nc.scalar.copy(OFl[:], outL[0:1, 16:16 * (T + 1)].rearrange("p (t b) -> p t b", t=T))
        OFu = lstm_pool.tile([1, T, 16], F32)
        nc.scalar.copy(OFu[:], outUO[0:1, 16:16 * (T + 1)].rearrange("p (t b) -> p t b", t=T))
        nc.sync.dma_start(d_out.ap()[0:1], OFl[:])
        nc.sync.dma_start(d_out.ap()[1:2], OFu[:])

    nc.compile()
    return nc


# --------------------------------------------------------------------------
# entry point
# --------------------------------------------------------------------------

def _in_maps(P, T):
    shared = {k: P[k] for k in ["w0T", "cb0", "w1p", "cb1", "w2T", "cb2", "w3T", "cb3",
                                "w4R", "cb4", "pwT", "pb", "wR4", "mr2", "m0neg", "hwT",
                                "hbl", "hbu", "i128"]}
    oinit = np.zeros((2, 16 * (T + 1)), BF_NP)
    oinit[1, :] = 1.0
    shared["oinit"] = oinit
    in_maps = []
    for c in range(NCORES):
        m = dict(shared)
        m["t0"] = np.ascontiguousarray(P["t0_full"][:, BC * c:BC * c + BC, :])
        in_maps.append(m)
    return in_maps


def kernel(**inputs):
    T = int(np.asarray(inputs["num_steps"]))
    if T not in _CACHE:
        _CACHE[T] = _build(T)
    nc = _CACHE[T]
    P = _prep(inputs)
    in_maps = _in_maps(P, T)
    res = run_bass_kernel_spmd(nc, in_maps, list(range(NCORES)))
    out = np.empty((128, T, 2), np.float32)
    for c in range(NCORES):
        out[BC * c:BC * c + BC] = res.results[c]["out"].transpose(2, 1, 0)
    return out
